# revision 23
# baseline (speedup 1.0000x reference)
"""Trainium2 Bass kernel for nn_GAT_KDE_14766097563859.

2-layer GAT over a 2048-node graph + per-(graph,layer) KDE soft-quantile
readouts. SPMD over 8 NeuronCores: GAT replicated, KDE sharded by feature dim
(each core owns 16 of 128 dims for all 12 (layer,graph) units = 192 slots).
Host does only input sharding + the tiny final linear heads.
"""
import sys
sys.path.insert(0, "/opt/trn_rl_repo")
import numpy as np

import concourse.bass as bass
import concourse.bacc as bacc
import concourse.mybir as mybir
from concourse.tile import TileContext
from concourse.masks import make_identity

F32 = mybir.dt.float32
F32R = mybir.dt.float32r
I32 = mybir.dt.int32
AF = mybir.ActivationFunctionType
ALU = mybir.AluOpType
AX = mybir.AxisListType

G, NG, N, E = 4, 512, 2048, 32768
IN_DIM, HID, HEADS, HC, OUT_DIM = 128, 32, 4, 128, 32
N_LAYERS, GRID, Q, NEG_SLOPE = 2, 500, 20, 0.2
N_CORES = 8
UNITS = (N_LAYERS + 1) * G            # 12, unit u = l*G + g
DPC = 16                              # dims per core
SLOTS = UNITS * DPC                   # 192
ITILES = [(0, 128), (128, 128), (256, 128), (384, 116)]
GW = 64                               # gather ring (chunks)
EGRP = 32                             # edge chunk group


def _edge_prep(edge_index):
    src = edge_index[0].astype(np.int64)
    dst = edge_index[1].astype(np.int64)
    s_all = np.concatenate([src, np.arange(N)])
    d_all = np.concatenate([dst, np.arange(N)])
    order = np.argsort(d_all, kind="stable")
    s_s, d_s = s_all[order], d_all[order]
    cs, cd, ct, cm = [], [], [], []
    for t in range(16):
        sel = (d_s // 128) == t
        se, de = s_s[sel], d_s[sel]
        ne = len(se)
        npad = (-ne) % 128
        se = np.concatenate([se, np.zeros(npad, np.int64)])
        de = np.concatenate([de, np.full(npad, t * 128, np.int64)])
        rm = np.concatenate([np.ones(ne, bool), np.zeros(npad, bool)])
        for c0 in range(0, len(se), 128):
            cs.append(se[c0:c0 + 128]); cd.append(de[c0:c0 + 128])
            cm.append(rm[c0:c0 + 128]); ct.append(t)
    C = len(cs)
    s_idx = np.stack(cs, 1).astype(np.int32)                  # [128, C]
    ohd_g = np.zeros((C, 128, 128), np.float32)
    ohrev = np.zeros((C, 128, 128), np.float32)
    for c in range(C):
        dl = (cd[c] - ct[c] * 128).astype(np.int64)
        ohd_g[c, dl, np.arange(128)] = 1.0
        rm = cm[c]
        ohrev[c, np.arange(128)[rm], dl[rm]] = 1.0
    return s_idx, np.asarray(ct, np.int32), ohd_g, ohrev


def build_program(C, chunk_tile, reps=1):
    nc = bacc.Bacc(None, target_bir_lowering=False, debug=True)

    xT_in = nc.declare_dram_parameter("xT", [128, N], F32, isOutput=False)
    Wi = [nc.declare_dram_parameter(f"W{l}", [128, HC], F32, isOutput=False) for l in range(2)]
    Avi = [nc.declare_dram_parameter(f"Av{l}", [128, 8], F32, isOutput=False) for l in range(2)]
    bri = [nc.declare_dram_parameter(f"brep{l}", [128, HC], F32, isOutput=False) for l in range(2)]
    sel_in = nc.declare_dram_parameter("sel", [128, DPC], F32, isOutput=False)
    iota3_in = nc.declare_dram_parameter("iota3", [3, GRID], F32, isOutput=False)
    tri_in = nc.declare_dram_parameter("tri", [GRID, GRID], F32, isOutput=False)
    sidx_in = nc.declare_dram_parameter("sidx", [128, C], I32, isOutput=False)
    repq_in = nc.declare_dram_parameter("repq", [4, 4, DPC, 128], F32, isOutput=False)
    repx_in = nc.declare_dram_parameter("repx", [4, 3, DPC, 128], F32R, isOutput=False)
    ohdg_in = nc.declare_dram_parameter("ohdg", [C, 128, 128], F32R, isOutput=False)
    ohrev_in = nc.declare_dram_parameter("ohrev", [C, 128, 128], F32R, isOutput=False)

    qv1_out = nc.declare_dram_parameter("qv1", [128, Q], F32, isOutput=True)
    qv2_out = nc.declare_dram_parameter("qv2", [64, Q], F32, isOutput=True)
    pmean_out = nc.declare_dram_parameter("pmean", [128, UNITS], F32, isOutput=True)
    pmax_out = nc.declare_dram_parameter("pmax", [128, UNITS], F32, isOutput=True)
    import os as _os
    KDBG = _os.environ.get("KDEBUG") == "1"
    if KDBG:
        dbg_dens = [nc.declare_dram_parameter(f"dbg_dens{j}", [128, SLOTS], F32, isOutput=True) for j in range(4)]
        dbg_L = nc.declare_dram_parameter("dbg_L", [128, GRID], F32, isOutput=True)
        dbg_X = nc.declare_dram_parameter("dbg_X", [128, 512], F32, isOutput=True)
        dbg_cur1 = nc.declare_dram_parameter("dbg_cur1", [128, N], F32, isOutput=True)
        dbg_p = nc.declare_dram_parameter("dbg_p", [128, ((C + 127) // 128) * 512], F32, isOutput=True)
        dbg_den = nc.declare_dram_parameter("dbg_den", [128, 64], F32, isOutput=True)
        dbg_cur2 = nc.declare_dram_parameter("dbg_cur2", [128, N], F32, isOutput=True)

    xh_al_hbm = nc.dram_tensor("xh_al", [N, 132], F32)

    with TileContext(nc) as tc:
        with (
            tc.tile_pool(name="cst", bufs=1) as cst,
            tc.tile_pool(name="wrk", bufs=2) as sb,
            tc.tile_pool(name="ps", bufs=1, space="PSUM") as ps,
        ):
            # ---------------- resident state ----------------
            curT = [cst.tile([128, N], F32, tag=f"curT{l}", name=f"curT{l}") for l in range(3)]
            W = [cst.tile([128, HC], F32, tag=f"Wt{l}", name=f"Wt{l}") for l in range(2)]
            Av = [cst.tile([128, 8], F32, tag=f"Avt{l}", name=f"Avt{l}") for l in range(2)]
            br = [cst.tile([128, HC], F32, tag=f"brt{l}", name=f"brt{l}") for l in range(2)]
            sel = cst.tile([128, DPC], F32)
            iota3 = cst.tile([3, GRID], F32)
            tri = cst.tile([128, 4, GRID], F32)
            sidx = cst.tile([128, C], I32)
            densT = [cst.tile([128, SLOTS], F32, tag=f"densT{j}", name=f"densT{j}") for j in range(4)]
            gridpk = [cst.tile([128, GRID], F32, tag="gpk0", name="gpk0"),
                      cst.tile([64, GRID], F32, tag="gpk1", name="gpk1")]
            S0 = [cst.tile([128, Q], F32, tag="S0a", name="S0a"), cst.tile([64, Q], F32, tag="S0b", name="S0b")]
            S1 = [cst.tile([128, Q], F32, tag="S1a", name="S1a"), cst.tile([64, Q], F32, tag="S1b", name="S1b")]
            pmean = cst.tile([128, UNITS], F32)
            pmax = cst.tile([128, UNITS], F32)
            id128 = cst.tile([128, 128], F32)
            gwin = cst.tile([128, GW, 132], F32)
            p_r = cst.tile([128, ((C + 127) // 128) * 512], F32R)
            xhT = cst.tile([128, N], F32)
            repq = cst.tile([DPC, 4, 4, 128], F32)
            repx = cst.tile([DPC, 4, 3, 128], F32R)
            ones16 = cst.tile([DPC, 512], F32R)
            al = cst.tile([128, 16, 8], F32)
            alr = cst.tile([128, 16, 8], F32R)
            stage = cst.tile([128, 16, 132], F32)

            make_identity(nc, id128[:])
            nc.gpsimd.dma_start(out=curT[0][:], in_=xT_in[:])
            for l in range(2):
                nc.gpsimd.dma_start(out=W[l][:], in_=Wi[l][:])
                nc.gpsimd.dma_start(out=Av[l][:], in_=Avi[l][:])
                nc.gpsimd.dma_start(out=br[l][:], in_=bri[l][:])
            nc.gpsimd.dma_start(out=sel[:], in_=sel_in[:])
            nc.gpsimd.dma_start(out=iota3[:], in_=iota3_in[:])
            for _j, (_i0, _isz) in enumerate(ITILES):
                nc.gpsimd.dma_start(out=tri[:_isz, _j, :], in_=tri_in[_i0:_i0 + _isz, :])
            nc.gpsimd.dma_start(out=sidx[:], in_=sidx_in[:])
            nc.gpsimd.dma_start(out=repq[:], in_=repq_in[:].rearrange("m v k p -> k m v p"))
            nc.gpsimd.dma_start(out=repx[:], in_=repx_in[:].rearrange("m v k p -> k m v p"))
            nc.gpsimd.memset(ones16[:].bitcast(F32), 1.0)

            # ---------------- KDE readout ----------------
            def readout(l):
                cur = curT[l]
                for g in range(G):
                    u = l * G + g
                    blk = cur[:, 512 * g:512 * (g + 1)]
                    pcopy = sb.tile([128, 512], F32, tag="pcopy")
                    nc.scalar.activation(pcopy[:], blk, AF.Copy,
                                         accum_out=pmean[:, u:u + 1])
                    nc.vector.tensor_reduce(out=pmax[:, u:u + 1], in_=blk,
                                            axis=AX.X, op=ALU.max)
                    ps_xs = ps.tile([DPC, 512], F32, tag="ps_m", bufs=2)
                    nc.tensor.matmul(ps_xs[:], sel[:], blk, start=True, stop=True)
                    xs = sb.tile([DPC, 512], F32, tag="xs")
                    nc.vector.tensor_copy(xs[:], ps_xs[:])
                    # stats
                    mn = sb.tile([DPC, 1], F32, tag="mn")
                    mx = sb.tile([DPC, 1], F32, tag="mx")
                    sm = sb.tile([DPC, 1], F32, tag="sm")
                    sq = sb.tile([DPC, 1], F32, tag="sq")
                    nc.vector.tensor_reduce(out=mn[:], in_=xs[:], axis=AX.X, op=ALU.min)
                    nc.vector.tensor_reduce(out=mx[:], in_=xs[:], axis=AX.X, op=ALU.max)
                    sdump = sb.tile([DPC, 512], F32, tag="sdump")
                    nc.scalar.activation(sdump[:], xs[:], AF.Copy, accum_out=sm[:])
                    nc.scalar.activation(sdump[:], xs[:], AF.Square, accum_out=sq[:])
                    var = sb.tile([DPC, 1], F32, tag="var")
                    mean = sb.tile([DPC, 1], F32, tag="mean")
                    nc.vector.tensor_scalar(out=mean[:], in0=sm[:], scalar1=1.0 / 512,
                                            scalar2=None, op0=ALU.mult)
                    nc.vector.tensor_scalar(out=var[:], in0=sq[:], scalar1=1.0 / 512,
                                            scalar2=None, op0=ALU.mult)
                    m2 = sb.tile([DPC, 1], F32, tag="m2")
                    nc.vector.tensor_tensor(out=m2[:], in0=mean[:], in1=mean[:], op=ALU.mult)
                    nc.vector.tensor_tensor(out=var[:], in0=var[:], in1=m2[:], op=ALU.subtract)
                    std = sb.tile([DPC, 1], F32, tag="std")
                    nc.scalar.activation(std[:], var[:], AF.Sqrt)
                    h = sb.tile([DPC, 1], F32, tag="h")
                    nc.vector.tensor_scalar(out=h[:], in0=std[:], scalar1=float(1e-8 / 3),
                                            scalar2=float(1.06 * 512 ** -0.2),
                                            op0=ALU.add, op1=ALU.mult)
                    rh = sb.tile([DPC, 1], F32, tag="rh")
                    nc.vector.reciprocal(out=rh[:], in_=h[:])
                    nc.vector.tensor_scalar(out=mn[:], in0=mn[:], scalar1=-1e-6, scalar2=None, op0=ALU.add)
                    nc.vector.tensor_scalar(out=mx[:], in0=mx[:], scalar1=1e-6, scalar2=None, op0=ALU.add)
                    dl = sb.tile([DPC, 1], F32, tag="dl")
                    nc.vector.tensor_tensor(out=dl[:], in0=mx[:], in1=mn[:], op=ALU.subtract)
                    nc.vector.tensor_scalar(out=dl[:], in0=dl[:], scalar1=1.0 / (GRID - 1), scalar2=None, op0=ALU.mult)
                    a0 = sb.tile([DPC, 1], F32, tag="a0")
                    a1 = sb.tile([DPC, 1], F32, tag="a1")
                    nc.vector.tensor_tensor(out=a0[:], in0=mn[:], in1=rh[:], op=ALU.mult)
                    nc.vector.tensor_tensor(out=a1[:], in0=dl[:], in1=rh[:], op=ALU.mult)
                    q0 = sb.tile([DPC, 1], F32, tag="q0")
                    q1 = sb.tile([DPC, 1], F32, tag="q1")
                    q2 = sb.tile([DPC, 1], F32, tag="q2")
                    nc.vector.tensor_tensor(out=q0[:], in0=a0[:], in1=a0[:], op=ALU.mult)
                    nc.vector.tensor_scalar(out=q0[:], in0=q0[:], scalar1=-0.5, scalar2=None, op0=ALU.mult)
                    nc.vector.tensor_tensor(out=q1[:], in0=a0[:], in1=a1[:], op=ALU.mult)
                    nc.vector.tensor_scalar(out=q1[:], in0=q1[:], scalar1=-1.0, scalar2=None, op0=ALU.mult)
                    nc.vector.tensor_tensor(out=q2[:], in0=a1[:], in1=a1[:], op=ALU.mult)
                    nc.vector.tensor_scalar(out=q2[:], in0=q2[:], scalar1=-0.5, scalar2=None, op0=ALU.mult)
                    # cstack [16, 12]: triples per row-type r:
                    # r0=(q0,q1,q2) r1=(a0,a1,0) r2=(1,0,0) r3=(mn,dl,0)
                    cstack = sb.tile([DPC, 12], F32, tag="cstack")
                    nc.gpsimd.memset(cstack[:], 0.0)
                    nc.vector.tensor_tensor(out=cstack[:, 0:1], in0=a0[:], in1=a0[:], op=ALU.mult)
                    nc.vector.tensor_scalar(out=cstack[:, 0:1], in0=cstack[:, 0:1], scalar1=-0.5, scalar2=None, op0=ALU.mult)
                    nc.vector.tensor_tensor(out=cstack[:, 1:2], in0=a0[:], in1=a1[:], op=ALU.mult)
                    nc.vector.tensor_scalar(out=cstack[:, 1:2], in0=cstack[:, 1:2], scalar1=-1.0, scalar2=None, op0=ALU.mult)
                    nc.vector.tensor_tensor(out=cstack[:, 2:3], in0=a1[:], in1=a1[:], op=ALU.mult)
                    nc.vector.tensor_scalar(out=cstack[:, 2:3], in0=cstack[:, 2:3], scalar1=-0.5, scalar2=None, op0=ALU.mult)
                    nc.vector.tensor_copy(cstack[:, 3:4], a0[:])
                    nc.vector.tensor_copy(cstack[:, 4:5], a1[:])
                    nc.gpsimd.memset(cstack[:, 6:7], 1.0)
                    nc.vector.tensor_copy(cstack[:, 9:10], mn[:])
                    nc.vector.tensor_copy(cstack[:, 10:11], dl[:])
                    # X rows [16, 512]
                    xh_s = sb.tile([DPC, 512], F32R, tag="xh_s")
                    nc.vector.tensor_tensor(out=xh_s[:], in0=xs[:],
                                            in1=rh[:].to_broadcast([DPC, 512]), op=ALU.mult)
                    xsq = sb.tile([DPC, 512], F32R, tag="xsq")
                    nc.vector.tensor_tensor(out=xsq[:], in0=xh_s[:], in1=xh_s[:], op=ALU.mult)
                    nc.vector.tensor_scalar(out=xsq[:], in0=xsq[:], scalar1=-0.5,
                                            scalar2=None, op0=ALU.mult)
                    Lq = []
                    X4 = []
                    for m in range(4):
                        # lh [3, 128]: quad coefs at cols 32s+r (r<3), grid coefs at 32s+3
                        ps_lh = ps.tile([3, 128], F32, tag="ps_m", bufs=2)
                        for r in range(4):
                            nc.tensor.matmul(ps_lh[:], cstack[:, 3 * r:3 * r + 3],
                                             repq[:, m, r, :],
                                             start=(r == 0), stop=(r == 3),
                                             skip_group_check=True)
                        lh = sb.tile([3, 128], F32, tag="lh", bufs=2)
                        nc.vector.tensor_copy(lh[:], ps_lh[:])
                        ps_L = ps.tile([128, GRID], F32, tag="ps_m", bufs=2)
                        nc.tensor.matmul(ps_L[:], lh[:], iota3[:], start=True, stop=True)
                        Lqm = sb.tile([128, GRID], F32R, tag=f"Lq{m}", name=f"Lqm{m}")
                        nc.vector.tensor_copy(Lqm[:], ps_L[:])
                        Lq.append(Lqm)
                        base = u * DPC + 4 * m
                        gp = gridpk[0] if base < 128 else gridpk[1]
                        gb = base % 128
                        nc.sync.dma_start(
                            out=gp[gb:gb + 4, :],
                            in_=Lqm[:].rearrange("(s b) i -> s b i", b=32)[:, 3:4, :].rearrange("s () i -> s i").bitcast(F32))
                        # X4m via f32r rep-MMs: rows 32s+0/3 = 1, 32s+1 = xh, 32s+2 = xsq
                        ps_X = ps.tile([128, 512], F32, tag="psu", bufs=2)
                        nc.tensor.matmul(ps_X[:], repx[:, m, 0, :].rearrange("k p -> k p"),
                                         ones16[:], start=True, stop=False, skip_group_check=True)
                        nc.tensor.matmul(ps_X[:], repx[:, m, 1, :], xh_s[:],
                                         start=False, stop=False, skip_group_check=True)
                        nc.tensor.matmul(ps_X[:], repx[:, m, 2, :], xsq[:],
                                         start=False, stop=True, skip_group_check=True)
                        X4m = sb.tile([128, 512], F32R, tag=f"X4{m}", name=f"X4m{m}")
                        nc.vector.tensor_copy(X4m[:], ps_X[:])
                        X4.append(X4m)
                    # slot loop
                    for k in range(DPC):
                        m, s = divmod(k, 4)
                        slot = u * DPC + k
                        for j, (i0, isz) in enumerate(ITILES):
                            psu = ps.tile([128, 512], F32, tag="psu", bufs=2)
                            nc.tensor.matmul(psu[:isz, :], Lq[m][32 * s:32 * s + 3, i0:i0 + isz],
                                             X4[m][32 * s:32 * s + 3, :], start=True, stop=True,
                                             tile_position=(32 * s, 0))
                            dump = sb.tile([128, 512], F32, tag="dump", bufs=3)
                            nc.scalar.activation(dump[:isz, :], psu[:isz, :], AF.Exp,
                                                 accum_out=densT[j][:isz, slot:slot + 1])

            # ---------------- quantile stage ----------------
            def quantiles():
                for half, npart in ((0, 128), (1, 64)):
                    s0c = 128 * half
                    cdf = ps.tile([npart, GRID], F32, tag="psu", bufs=2)
                    for cg in range(npart // 32):
                        for j, (i0, isz) in enumerate(ITILES):
                            nc.tensor.matmul(cdf[32 * cg:32 * cg + 32, :],
                                             densT[j][:isz, s0c + 32 * cg:s0c + 32 * cg + 32],
                                             tri[:isz, j, :],
                                             start=(j == 0), stop=(j == 3),
                                             tile_position=(0, 32 * cg))
                    rec = sb.tile([npart, 1], F32, tag="rec")
                    nc.vector.reciprocal(out=rec[:], in_=cdf[:, GRID - 1:GRID])
                    cdfn = sb.tile([npart, GRID], F32, tag="cdfn")
                    nc.vector.tensor_tensor(out=cdfn[:], in0=cdf[:],
                                            in1=rec[:].to_broadcast([npart, GRID]), op=ALU.mult)
                    gp = gridpk[half][0:npart, :]
                    qs = np.linspace(0.0, 1.0, Q)
                    for qi in range(Q):
                        d1 = sb.tile([npart, GRID], F32, tag="d1")
                        nc.vector.tensor_scalar(out=d1[:], in0=cdfn[:],
                                                scalar1=float(-qs[qi]), scalar2=None, op0=ALU.add)
                        nc.vector.tensor_scalar(out=d1[:].bitcast(I32), in0=d1[:].bitcast(I32),
                                                scalar1=0x7FFFFFFF, scalar2=None, op0=ALU.bitwise_and)
                        w = sb.tile([npart, GRID], F32, tag="w", bufs=3)
                        nc.scalar.activation(w[:], d1[:], AF.Sigmoid, scale=-100.0,
                                             accum_out=S0[half][0:npart, qi:qi + 1])
                        gw_t = sb.tile([npart, GRID], F32, tag="gw", bufs=3)
                        nc.vector.tensor_tensor(out=gw_t[:], in0=w[:], in1=gp, op=ALU.mult)
                        nc.scalar.activation(gw_t[:], gw_t[:], AF.Copy,
                                             accum_out=S1[half][0:npart, qi:qi + 1])
                    r0 = sb.tile([npart, Q], F32, tag="r0")
                    nc.vector.tensor_scalar(out=r0[:], in0=S0[half][0:npart, :],
                                            scalar1=1e-8, scalar2=None, op0=ALU.add)
                    nc.vector.reciprocal(out=r0[:], in_=r0[:])
                    qv = sb.tile([npart, Q], F32, tag="qv")
                    nc.vector.tensor_tensor(out=qv[:], in0=S1[half][0:npart, :],
                                            in1=r0[:], op=ALU.mult)
                    nc.sync.dma_start(out=(qv1_out if half == 0 else qv2_out)[:], in_=qv[:])

            # ---------------- GAT layer ----------------
            def finish_tile(l, t, raw, denps):
                rd = sb.tile([128, 4], F32, tag="rd")
                nc.vector.tensor_scalar(out=rd[:], in0=denps[:, 4 * t:4 * t + 4],
                                        scalar1=1e-16, scalar2=None, op0=ALU.add)
                nc.vector.reciprocal(out=rd[:], in_=rd[:])
                o = sb.tile([128, HC], F32, tag="otile")
                nc.vector.tensor_tensor(
                    out=o[:].rearrange("p (h c2) -> p h c2", h=4),
                    in0=raw[:].rearrange("p (h c2) -> p h c2", h=4),
                    in1=rd[:].rearrange("p h -> p h ()").to_broadcast([128, 4, 32]),
                    op=ALU.mult)
                nc.vector.tensor_tensor(out=o[:], in0=o[:], in1=br[l][:], op=ALU.add)
                if l == 0:
                    nc.vector.tensor_scalar(out=o[:], in0=o[:], scalar1=0.0,
                                            scalar2=None, op0=ALU.max)
                pt = ps.tile([128, 128], F32, tag="ps_m", bufs=2)
                nc.tensor.transpose(pt[:], o[:], id128[:])
                nc.vector.tensor_copy(curT[l + 1][:, 128 * t:128 * (t + 1)], pt[:])

            def gat_layer(l):
                cur = curT[l]
                for b in range(4):
                    pxh = ps.tile([128, 512], F32, tag="ps_m", bufs=2)
                    nc.tensor.matmul(pxh[:], W[l][:], cur[:, 512 * b:512 * (b + 1)],
                                     start=True, stop=True)
                    nc.vector.tensor_copy(xhT[:, 512 * b:512 * (b + 1)], pxh[:])
                for t in range(16):
                    pal = ps.tile([128, 8], F32, tag="ps_m", bufs=2)
                    nc.tensor.matmul(pal[:], xhT[:, 128 * t:128 * (t + 1)], Av[l][:],
                                     start=True, stop=True)
                    nc.vector.tensor_copy(al[:, t, :], pal[:])
                    pxr = ps.tile([128, 128], F32, tag="ps_m", bufs=2)
                    nc.tensor.transpose(pxr[:], xhT[:, 128 * t:128 * (t + 1)], id128[:])
                    nc.vector.tensor_copy(stage[:, t, 0:128], pxr[:])
                nc.vector.tensor_copy(alr[:], al[:])
                nc.vector.tensor_copy(stage[:, :, 128:132], al[:, :, 0:4])
                nc.sync.dma_start(out=xh_al_hbm[:].rearrange("(t p) d -> p t d", p=128),
                                  in_=stage[:])
                denps = ps.tile([128, 64], F32, tag="denps")
                raw = None
                cur_t = -1
                for g0 in range(0, C, EGRP):
                    cn = min(EGRP, C - g0)
                    evps = ps.tile([128, 4 * EGRP], F32, tag="evps", bufs=1)
                    for ci in range(cn):
                        c = g0 + ci
                        ohg = sb.tile([128, 128], F32R, tag="ohg", bufs=6)
                        nc.sync.dma_start(out=ohg[:], in_=ohdg_in[c, :, :])
                        nc.tensor.matmul(evps[:, 4 * ci:4 * ci + 4], ohg[:],
                                         alr[:, int(chunk_tile[c]), 4:8],
                                         start=True, stop=True)
                        nc.gpsimd.indirect_dma_start(
                            out=gwin[:, c % GW, :], out_offset=None, in_=xh_al_hbm[:],
                            in_offset=bass.IndirectOffsetOnAxis(ap=sidx[:, c:c + 1], axis=0))
                    r0w = g0 % GW
                    z = sb.tile([128, EGRP, 4], F32, tag="z", bufs=2)
                    nc.vector.tensor_tensor(
                        out=z[:, 0:cn, :],
                        in0=evps[:, 0:4 * cn].rearrange("p (c f) -> p c f", f=4),
                        in1=gwin[:, r0w:r0w + cn, 128:132], op=ALU.add)
                    zs = sb.tile([128, EGRP, 4], F32, tag="zs", bufs=2)
                    nc.vector.tensor_scalar(out=zs[:, 0:cn, :], in0=z[:, 0:cn, :],
                                            scalar1=NEG_SLOPE, scalar2=None, op0=ALU.mult)
                    nc.vector.tensor_tensor(out=z[:, 0:cn, :], in0=z[:, 0:cn, :],
                                            in1=zs[:, 0:cn, :], op=ALU.max)
                    nc.scalar.activation(
                        p_r[:, 4 * g0:4 * (g0 + cn)].rearrange("p (c f) -> p c f", f=4),
                        z[:, 0:cn, :], AF.Exp)
                    for ci in range(cn):
                        c = g0 + ci
                        t = int(chunk_tile[c])
                        first = (c == 0) or (int(chunk_tile[c - 1]) != t)
                        last = (c == C - 1) or (int(chunk_tile[c + 1]) != t)
                        if first:
                            if raw is not None:
                                finish_tile(l, cur_t, raw, denps)
                            raw = ps.tile([128, HC], F32, tag="raw",
                                          padded_shape=[128, 512], bufs=2)
                            cur_t = t
                        ohv = sb.tile([128, 128], F32R, tag="ohv", bufs=6)
                        nc.sync.dma_start(out=ohv[:], in_=ohrev_in[c, :, :])
                        nc.tensor.matmul(denps[:, 4 * t:4 * t + 4], ohv[:],
                                         p_r[:, 4 * c:4 * c + 4],
                                         start=first, stop=last, skip_group_check=True)
                        sxh = sb.tile([128, HC], F32R, tag="sxh", bufs=3)
                        nc.vector.tensor_tensor(
                            out=sxh[:].rearrange("p (h c2) -> p h c2", h=4),
                            in0=gwin[:, c % GW, 0:128].rearrange("p (h c2) -> p h c2", h=4),
                            in1=p_r[:, 4 * c:4 * c + 4].rearrange("p h -> p h ()").to_broadcast([128, 4, 32]),
                            op=ALU.mult)
                        nc.tensor.matmul(raw[:], ohv[:], sxh[:],
                                         start=first, stop=last, skip_group_check=True)
                finish_tile(l, cur_t, raw, denps)
                if KDBG and l == 0:
                    nc.sync.dma_start(out=dbg_p[:], in_=p_r[:].bitcast(F32))
                    dcp = sb.tile([128, 64], F32, tag="dcp")
                    nc.vector.tensor_copy(dcp[:], denps[:])
                    nc.sync.dma_start(out=dbg_den[:], in_=dcp[:])

            # ---------------- main ----------------
            import os
            phases = os.environ.get("KPHASES", "all")
            if phases != "all":
                nc.gpsimd.memset(gridpk[0][:], 0.0)
                nc.gpsimd.memset(gridpk[1][:], 0.0)
                for j in range(4):
                    nc.gpsimd.memset(densT[j][:], 0.0)
                nc.gpsimd.memset(curT[1][:], 0.0)
                nc.gpsimd.memset(curT[2][:], 0.0)
            for _ in range(reps):
                if phases == "r0":
                    readout(0); quantiles()
                elif phases == "r0g0":
                    readout(0); gat_layer(0); quantiles()
                else:
                    readout(0)
                    gat_layer(0)
                    readout(1)
                    gat_layer(1)
                    readout(2)
                    quantiles()
            nc.sync.dma_start(out=pmean_out[:], in_=pmean[:])
            nc.sync.dma_start(out=pmax_out[:], in_=pmax[:])
            if KDBG:
                for j in range(4):
                    nc.sync.dma_start(out=dbg_dens[j][:], in_=densT[j][:])
                nc.sync.dma_start(out=dbg_cur1[:], in_=curT[1][:])
                nc.sync.dma_start(out=dbg_cur2[:], in_=curT[2][:])
    nc.compile()
    return nc


_CACHE = {}


def _get_program(C, chunk_tile, reps=1):
    import os; key = (C, tuple(chunk_tile.tolist()), reps, os.environ.get("KPHASES", "all"), os.environ.get("KDEBUG"))
    if key not in _CACHE:
        _CACHE[key] = build_program(C, chunk_tile, reps)
    return _CACHE[key]


def _host_inputs(inputs, s_idx, C, ohd_g, ohrev):
    x = np.asarray(inputs["x"], np.float32)
    repq = np.zeros((4, 4, DPC, 128), np.float32)
    repx = np.zeros((4, 3, DPC, 128), np.float32)
    for m in range(4):
        for s in range(4):
            k = 4 * m + s
            for r in range(4):
                repq[m, r, k, 32 * s + r] = 1.0
            repx[m, 0, k, 32 * s + 0] = 1.0
            repx[m, 0, k, 32 * s + 3] = 1.0
            repx[m, 1, k, 32 * s + 1] = 1.0
            repx[m, 2, k, 32 * s + 2] = 1.0
    im_base = dict(
        repq=repq, repx=repx,
        xT=np.ascontiguousarray(x.T),
        sidx=s_idx,
        ohdg=ohd_g, ohrev=ohrev,
        iota3=np.stack([np.ones(GRID), np.arange(GRID), np.arange(GRID) ** 2]).astype(np.float32),
        tri=np.triu(np.ones((GRID, GRID), np.float32)),  # U[j,i]=1 if j<=i
    )
    for l in range(2):
        A = np.zeros((128, 8), np.float32)
        as_l = np.asarray(inputs[f"as{l}"], np.float32)
        ad_l = np.asarray(inputs[f"ad{l}"], np.float32)
        for h in range(HEADS):
            A[h * HID:(h + 1) * HID, h] = as_l[h]
            A[h * HID:(h + 1) * HID, 4 + h] = ad_l[h]
        im_base[f"W{l}"] = np.asarray(inputs[f"W{l}"], np.float32)
        im_base[f"Av{l}"] = A
        im_base[f"brep{l}"] = np.tile(np.asarray(inputs[f"b{l}"], np.float32)[None, :], (128, 1))
    in_maps = []
    for c in range(N_CORES):
        selm = np.zeros((128, DPC), np.float32)
        for k in range(DPC):
            selm[DPC * c + k, k] = 1.0
        in_maps.append({**im_base, "sel": selm})
    return in_maps


def kernel(**inputs) -> np.ndarray:
    from concourse.bass_utils import run_bass_kernel_spmd
    s_idx, chunk_tile, ohd_g, ohrev = _edge_prep(np.asarray(inputs["edge_index"]))
    C = s_idx.shape[1]
    nc = _get_program(C, chunk_tile)
    in_maps = _host_inputs(inputs, s_idx, C, ohd_g, ohrev)
    res = run_bass_kernel_spmd(nc, in_maps, list(range(N_CORES))).results
    return _assemble(inputs, res)


def _assemble(inputs, res):
    # kf[l, g, d, q]
    kf = np.zeros((3, G, 128, Q), np.float64)
    for c in range(N_CORES):
        qv = np.concatenate([res[c]["qv1"], res[c]["qv2"]], 0)  # [192, Q]
        for u in range(UNITS):
            l, g = divmod(u, G)
            kf[l, g, DPC * c:DPC * (c + 1), :] = qv[u * DPC:(u + 1) * DPC, :]
    pmean = res[0]["pmean"] / 512.0          # [128, 12]
    pmax = res[0]["pmax"]
    pool_w = np.asarray(inputs["pool_w"], np.float64)
    beta = np.asarray(inputs["beta"], np.float64)
    h0 = float(np.asarray(inputs["h0"]).reshape(-1)[0])
    h_list, k_list = [], []
    for l in range(3):
        wp = (pool_w[0] * pmean[:, l * G:(l + 1) * G] + pool_w[1] * pmax[:, l * G:(l + 1) * G]).T  # [G, 128]
        lpW = np.asarray(inputs[f"lpW{l}"], np.float64)
        lpb = np.asarray(inputs[f"lpb{l}"], np.float64)
        h_list.append(wp @ lpW + lpb)
        kW = np.asarray(inputs[f"kW{l}"], np.float64)
        kb = np.asarray(inputs[f"kb{l}"], np.float64)
        k_list.append(kf[l].reshape(G, 128 * Q) @ kW + kb)
    main_out = np.mean(h_list, axis=0)
    kde_out = np.mean(k_list, axis=0)
    risk = (main_out + kde_out) @ beta + h0
    return risk.astype(np.float32)


# revision 25
# speedup vs baseline: 2.1619x; 2.1619x over previous
"""Trainium2 Bass kernel for nn_GAT_KDE_14766097563859.

2-layer GAT over a 2048-node graph + per-(graph,layer) KDE soft-quantile
readouts. SPMD over 8 NeuronCores: GAT replicated, KDE sharded by feature dim
(each core owns 16 of 128 dims for all 12 (layer,graph) units = 192 slots).
Host does only input sharding + the tiny final linear heads.
"""
import sys
sys.path.insert(0, "/opt/trn_rl_repo")
import numpy as np

import concourse.bass as bass
import concourse.bacc as bacc
import concourse.mybir as mybir
from concourse.tile import TileContext
from concourse.masks import make_identity

F32 = mybir.dt.float32
F32R = mybir.dt.float32r
I32 = mybir.dt.int32
AF = mybir.ActivationFunctionType
ALU = mybir.AluOpType
AX = mybir.AxisListType

G, NG, N, E = 4, 512, 2048, 32768
IN_DIM, HID, HEADS, HC, OUT_DIM = 128, 32, 4, 128, 32
N_LAYERS, GRID, Q, NEG_SLOPE = 2, 500, 20, 0.2
N_CORES = 8
UNITS = (N_LAYERS + 1) * G            # 12, unit u = l*G + g
DPC = 16                              # dims per core
SLOTS = UNITS * DPC                   # 192
ITILES = [(0, 128), (128, 128), (256, 128), (384, 116)]
GW = 64                               # gather ring (chunks)
EGRP = 32                             # edge chunk group


def _edge_prep(edge_index):
    src = edge_index[0].astype(np.int64)
    dst = edge_index[1].astype(np.int64)
    s_all = np.concatenate([src, np.arange(N)])
    d_all = np.concatenate([dst, np.arange(N)])
    order = np.argsort(d_all, kind="stable")
    s_s, d_s = s_all[order], d_all[order]
    cs, cd, ct, cm = [], [], [], []
    for t in range(16):
        sel = (d_s // 128) == t
        se, de = s_s[sel], d_s[sel]
        ne = len(se)
        npad = (-ne) % 128
        se = np.concatenate([se, np.zeros(npad, np.int64)])
        de = np.concatenate([de, np.full(npad, t * 128, np.int64)])
        rm = np.concatenate([np.ones(ne, bool), np.zeros(npad, bool)])
        for c0 in range(0, len(se), 128):
            cs.append(se[c0:c0 + 128]); cd.append(de[c0:c0 + 128])
            cm.append(rm[c0:c0 + 128]); ct.append(t)
    C = len(cs)
    s_idx = np.stack(cs, 1).astype(np.int32)                  # [128, C]
    ohd_g = np.zeros((C, 128, 128), np.float32)
    ohrev = np.zeros((C, 128, 128), np.float32)
    for c in range(C):
        dl = (cd[c] - ct[c] * 128).astype(np.int64)
        ohd_g[c, dl, np.arange(128)] = 1.0
        rm = cm[c]
        ohrev[c, np.arange(128)[rm], dl[rm]] = 1.0
    return s_idx, np.asarray(ct, np.int32), ohd_g, ohrev


def build_program(C, chunk_tile, reps=1):
    nc = bacc.Bacc(None, target_bir_lowering=False, debug=True)

    xT_in = nc.declare_dram_parameter("xT", [128, N], F32, isOutput=False)
    Wi = [nc.declare_dram_parameter(f"W{l}", [128, HC], F32, isOutput=False) for l in range(2)]
    Avi = [nc.declare_dram_parameter(f"Av{l}", [128, 8], F32, isOutput=False) for l in range(2)]
    bri = [nc.declare_dram_parameter(f"brep{l}", [128, HC], F32, isOutput=False) for l in range(2)]
    sel_in = nc.declare_dram_parameter("sel", [128, DPC], F32, isOutput=False)
    iota3_in = nc.declare_dram_parameter("iota3", [3, GRID], F32, isOutput=False)
    tri_in = nc.declare_dram_parameter("tri", [GRID, GRID], F32, isOutput=False)
    sidx_in = nc.declare_dram_parameter("sidx", [128, C], I32, isOutput=False)
    repq_in = nc.declare_dram_parameter("repq", [4, 4, DPC, 128], F32, isOutput=False)
    repx_in = nc.declare_dram_parameter("repx", [4, 3, DPC, 128], F32R, isOutput=False)
    ohdg_in = nc.declare_dram_parameter("ohdg", [C, 128, 128], F32R, isOutput=False)
    ohrev_in = nc.declare_dram_parameter("ohrev", [C, 128, 128], F32R, isOutput=False)

    qv1_out = nc.declare_dram_parameter("qv1", [128, Q], F32, isOutput=True)
    qv2_out = nc.declare_dram_parameter("qv2", [64, Q], F32, isOutput=True)
    pmean_out = nc.declare_dram_parameter("pmean", [128, UNITS], F32, isOutput=True)
    pmax_out = nc.declare_dram_parameter("pmax", [128, UNITS], F32, isOutput=True)
    import os as _os
    KDBG = _os.environ.get("KDEBUG") == "1"
    if KDBG:
        dbg_dens = [nc.declare_dram_parameter(f"dbg_dens{j}", [128, SLOTS], F32, isOutput=True) for j in range(4)]
        dbg_L = nc.declare_dram_parameter("dbg_L", [128, GRID], F32, isOutput=True)
        dbg_X = nc.declare_dram_parameter("dbg_X", [128, 512], F32, isOutput=True)
        dbg_cur1 = nc.declare_dram_parameter("dbg_cur1", [128, N], F32, isOutput=True)
        dbg_p = nc.declare_dram_parameter("dbg_p", [128, ((C + 127) // 128) * 512], F32, isOutput=True)
        dbg_den = nc.declare_dram_parameter("dbg_den", [128, 64], F32, isOutput=True)
        dbg_cur2 = nc.declare_dram_parameter("dbg_cur2", [128, N], F32, isOutput=True)

    xh_al_hbm = nc.dram_tensor("xh_al", [N, 132], F32)

    with TileContext(nc) as tc:
        with (
            tc.tile_pool(name="cst", bufs=1) as cst,
            tc.tile_pool(name="wrk", bufs=2) as sb,
            tc.tile_pool(name="ps", bufs=1, space="PSUM") as ps,
        ):
            # ---------------- resident state ----------------
            curT = [cst.tile([128, N], F32, tag=f"curT{l}", name=f"curT{l}") for l in range(3)]
            W = [cst.tile([128, HC], F32, tag=f"Wt{l}", name=f"Wt{l}") for l in range(2)]
            Av = [cst.tile([128, 8], F32, tag=f"Avt{l}", name=f"Avt{l}") for l in range(2)]
            br = [cst.tile([128, HC], F32, tag=f"brt{l}", name=f"brt{l}") for l in range(2)]
            sel = cst.tile([128, DPC], F32)
            iota3 = cst.tile([3, GRID], F32)
            tri = cst.tile([128, 4, GRID], F32)
            sidx = cst.tile([128, C], I32)
            densT = [cst.tile([128, SLOTS], F32, tag=f"densT{j}", name=f"densT{j}") for j in range(4)]
            gridpk = [cst.tile([128, GRID], F32, tag="gpk0", name="gpk0"),
                      cst.tile([64, GRID], F32, tag="gpk1", name="gpk1")]
            S0 = [cst.tile([128, Q], F32, tag="S0a", name="S0a"), cst.tile([64, Q], F32, tag="S0b", name="S0b")]
            S1 = [cst.tile([128, Q], F32, tag="S1a", name="S1a"), cst.tile([64, Q], F32, tag="S1b", name="S1b")]
            pmean = cst.tile([128, UNITS], F32)
            pmax = cst.tile([128, UNITS], F32)
            id128 = cst.tile([128, 128], F32)
            gwin = cst.tile([128, GW, 132], F32)
            p_r = cst.tile([128, ((C + 127) // 128) * 512], F32R)
            xhT = cst.tile([128, N], F32)
            repq = cst.tile([DPC, 4, 4, 128], F32)
            repx = cst.tile([DPC, 4, 3, 128], F32R)
            ones16 = cst.tile([DPC, 512], F32R)
            al = cst.tile([128, 16, 8], F32)
            alr = cst.tile([128, 16, 8], F32R)
            stage = cst.tile([128, 16, 132], F32)

            make_identity(nc, id128[:])
            nc.gpsimd.dma_start(out=curT[0][:], in_=xT_in[:])
            for l in range(2):
                nc.gpsimd.dma_start(out=W[l][:], in_=Wi[l][:])
                nc.gpsimd.dma_start(out=Av[l][:], in_=Avi[l][:])
                nc.gpsimd.dma_start(out=br[l][:], in_=bri[l][:])
            nc.gpsimd.dma_start(out=sel[:], in_=sel_in[:])
            nc.gpsimd.dma_start(out=iota3[:], in_=iota3_in[:])
            for _j, (_i0, _isz) in enumerate(ITILES):
                nc.gpsimd.dma_start(out=tri[:_isz, _j, :], in_=tri_in[_i0:_i0 + _isz, :])
            nc.gpsimd.dma_start(out=sidx[:], in_=sidx_in[:])
            nc.gpsimd.dma_start(out=repq[:], in_=repq_in[:].rearrange("m v k p -> k m v p"))
            nc.gpsimd.dma_start(out=repx[:], in_=repx_in[:].rearrange("m v k p -> k m v p"))
            nc.gpsimd.memset(ones16[:].bitcast(F32), 1.0)

            # ---------------- KDE readout ----------------
            def readout(l):
                cur = curT[l]
                for g in range(G):
                    u = l * G + g
                    blk = cur[:, 512 * g:512 * (g + 1)]
                    pcopy = sb.tile([128, 512], F32, tag="pcopy")
                    nc.scalar.activation(pcopy[:], blk, AF.Copy,
                                         accum_out=pmean[:, u:u + 1])
                    nc.vector.tensor_reduce(out=pmax[:, u:u + 1], in_=blk,
                                            axis=AX.X, op=ALU.max)
                    ps_xs = ps.tile([DPC, 512], F32, tag="ps_m", bufs=1)
                    nc.tensor.matmul(ps_xs[:], sel[:], blk, start=True, stop=True)
                    xs = sb.tile([DPC, 512], F32, tag="xs")
                    nc.vector.tensor_copy(xs[:], ps_xs[:])
                    # stats
                    mn = sb.tile([DPC, 1], F32, tag="mn")
                    mx = sb.tile([DPC, 1], F32, tag="mx")
                    sm = sb.tile([DPC, 1], F32, tag="sm")
                    sq = sb.tile([DPC, 1], F32, tag="sq")
                    nc.vector.tensor_reduce(out=mn[:], in_=xs[:], axis=AX.X, op=ALU.min)
                    nc.vector.tensor_reduce(out=mx[:], in_=xs[:], axis=AX.X, op=ALU.max)
                    sdump = sb.tile([DPC, 512], F32, tag="sdump")
                    nc.scalar.activation(sdump[:], xs[:], AF.Copy, accum_out=sm[:])
                    nc.scalar.activation(sdump[:], xs[:], AF.Square, accum_out=sq[:])
                    var = sb.tile([DPC, 1], F32, tag="var")
                    mean = sb.tile([DPC, 1], F32, tag="mean")
                    nc.vector.tensor_scalar(out=mean[:], in0=sm[:], scalar1=1.0 / 512,
                                            scalar2=None, op0=ALU.mult)
                    nc.vector.tensor_scalar(out=var[:], in0=sq[:], scalar1=1.0 / 512,
                                            scalar2=None, op0=ALU.mult)
                    m2 = sb.tile([DPC, 1], F32, tag="m2")
                    nc.vector.tensor_tensor(out=m2[:], in0=mean[:], in1=mean[:], op=ALU.mult)
                    nc.vector.tensor_tensor(out=var[:], in0=var[:], in1=m2[:], op=ALU.subtract)
                    std = sb.tile([DPC, 1], F32, tag="std")
                    nc.scalar.activation(std[:], var[:], AF.Sqrt)
                    h = sb.tile([DPC, 1], F32, tag="h")
                    nc.vector.tensor_scalar(out=h[:], in0=std[:], scalar1=float(1e-8 / 3),
                                            scalar2=float(1.06 * 512 ** -0.2),
                                            op0=ALU.add, op1=ALU.mult)
                    rh = sb.tile([DPC, 1], F32, tag="rh")
                    nc.vector.reciprocal(out=rh[:], in_=h[:])
                    nc.vector.tensor_scalar(out=mn[:], in0=mn[:], scalar1=-1e-6, scalar2=None, op0=ALU.add)
                    nc.vector.tensor_scalar(out=mx[:], in0=mx[:], scalar1=1e-6, scalar2=None, op0=ALU.add)
                    dl = sb.tile([DPC, 1], F32, tag="dl")
                    nc.vector.tensor_tensor(out=dl[:], in0=mx[:], in1=mn[:], op=ALU.subtract)
                    nc.vector.tensor_scalar(out=dl[:], in0=dl[:], scalar1=1.0 / (GRID - 1), scalar2=None, op0=ALU.mult)
                    a0 = sb.tile([DPC, 1], F32, tag="a0")
                    a1 = sb.tile([DPC, 1], F32, tag="a1")
                    nc.vector.tensor_tensor(out=a0[:], in0=mn[:], in1=rh[:], op=ALU.mult)
                    nc.vector.tensor_tensor(out=a1[:], in0=dl[:], in1=rh[:], op=ALU.mult)
                    q0 = sb.tile([DPC, 1], F32, tag="q0")
                    q1 = sb.tile([DPC, 1], F32, tag="q1")
                    q2 = sb.tile([DPC, 1], F32, tag="q2")
                    nc.vector.tensor_tensor(out=q0[:], in0=a0[:], in1=a0[:], op=ALU.mult)
                    nc.vector.tensor_scalar(out=q0[:], in0=q0[:], scalar1=-0.5, scalar2=None, op0=ALU.mult)
                    nc.vector.tensor_tensor(out=q1[:], in0=a0[:], in1=a1[:], op=ALU.mult)
                    nc.vector.tensor_scalar(out=q1[:], in0=q1[:], scalar1=-1.0, scalar2=None, op0=ALU.mult)
                    nc.vector.tensor_tensor(out=q2[:], in0=a1[:], in1=a1[:], op=ALU.mult)
                    nc.vector.tensor_scalar(out=q2[:], in0=q2[:], scalar1=-0.5, scalar2=None, op0=ALU.mult)
                    # cstack [16, 12]: triples per row-type r:
                    # r0=(q0,q1,q2) r1=(a0,a1,0) r2=(1,0,0) r3=(mn,dl,0)
                    cstack = sb.tile([DPC, 12], F32, tag="cstack")
                    nc.gpsimd.memset(cstack[:], 0.0)
                    nc.vector.tensor_tensor(out=cstack[:, 0:1], in0=a0[:], in1=a0[:], op=ALU.mult)
                    nc.vector.tensor_scalar(out=cstack[:, 0:1], in0=cstack[:, 0:1], scalar1=-0.5, scalar2=None, op0=ALU.mult)
                    nc.vector.tensor_tensor(out=cstack[:, 1:2], in0=a0[:], in1=a1[:], op=ALU.mult)
                    nc.vector.tensor_scalar(out=cstack[:, 1:2], in0=cstack[:, 1:2], scalar1=-1.0, scalar2=None, op0=ALU.mult)
                    nc.vector.tensor_tensor(out=cstack[:, 2:3], in0=a1[:], in1=a1[:], op=ALU.mult)
                    nc.vector.tensor_scalar(out=cstack[:, 2:3], in0=cstack[:, 2:3], scalar1=-0.5, scalar2=None, op0=ALU.mult)
                    nc.vector.tensor_copy(cstack[:, 3:4], a0[:])
                    nc.vector.tensor_copy(cstack[:, 4:5], a1[:])
                    nc.gpsimd.memset(cstack[:, 6:7], 1.0)
                    nc.vector.tensor_copy(cstack[:, 9:10], mn[:])
                    nc.vector.tensor_copy(cstack[:, 10:11], dl[:])
                    # X rows [16, 512]
                    xh_s = sb.tile([DPC, 512], F32R, tag="xh_s")
                    nc.vector.tensor_tensor(out=xh_s[:], in0=xs[:],
                                            in1=rh[:].to_broadcast([DPC, 512]), op=ALU.mult)
                    xsq = sb.tile([DPC, 512], F32R, tag="xsq")
                    nc.vector.tensor_tensor(out=xsq[:], in0=xh_s[:], in1=xh_s[:], op=ALU.mult)
                    nc.vector.tensor_scalar(out=xsq[:], in0=xsq[:], scalar1=-0.5,
                                            scalar2=None, op0=ALU.mult)
                    Lq = []
                    X4 = []
                    for m in range(4):
                        # lh [3, 128]: quad coefs at cols 32s+r (r<3), grid coefs at 32s+3
                        ps_lh = ps.tile([3, 128], F32, tag="ps_m", bufs=1)
                        for r in range(4):
                            nc.tensor.matmul(ps_lh[:], cstack[:, 3 * r:3 * r + 3],
                                             repq[:, m, r, :],
                                             start=(r == 0), stop=(r == 3),
                                             skip_group_check=True)
                        lh = sb.tile([3, 128], F32, tag="lh", bufs=2)
                        nc.vector.tensor_copy(lh[:], ps_lh[:])
                        ps_L = ps.tile([128, GRID], F32, tag="ps_m", bufs=1)
                        nc.tensor.matmul(ps_L[:], lh[:], iota3[:], start=True, stop=True)
                        Lqm = sb.tile([128, GRID], F32R, tag=f"Lq{m}", name=f"Lqm{m}")
                        nc.vector.tensor_copy(Lqm[:], ps_L[:])
                        Lq.append(Lqm)
                        base = u * DPC + 4 * m
                        gp = gridpk[0] if base < 128 else gridpk[1]
                        gb = base % 128
                        nc.sync.dma_start(
                            out=gp[gb:gb + 4, :],
                            in_=Lqm[:].rearrange("(s b) i -> s b i", b=32)[:, 3:4, :].rearrange("s () i -> s i").bitcast(F32))
                        # X4m via f32r rep-MMs: rows 32s+0/3 = 1, 32s+1 = xh, 32s+2 = xsq
                        ps_X = ps.tile([128, 512], F32, tag="psu", bufs=3)
                        nc.tensor.matmul(ps_X[:], repx[:, m, 0, :].rearrange("k p -> k p"),
                                         ones16[:], start=True, stop=False, skip_group_check=True)
                        nc.tensor.matmul(ps_X[:], repx[:, m, 1, :], xh_s[:],
                                         start=False, stop=False, skip_group_check=True)
                        nc.tensor.matmul(ps_X[:], repx[:, m, 2, :], xsq[:],
                                         start=False, stop=True, skip_group_check=True)
                        X4m = sb.tile([128, 512], F32R, tag=f"X4{m}", name=f"X4m{m}")
                        nc.vector.tensor_copy(X4m[:], ps_X[:])
                        X4.append(X4m)
                    # slot loop
                    for k in range(DPC):
                        m, s = divmod(k, 4)
                        slot = u * DPC + k
                        for j, (i0, isz) in enumerate(ITILES):
                            psu = ps.tile([128, 512], F32, tag="psu", bufs=3)
                            nc.tensor.matmul(psu[:isz, :], Lq[m][32 * s:32 * s + 3, i0:i0 + isz],
                                             X4[m][32 * s:32 * s + 3, :], start=True, stop=True,
                                             tile_position=(32 * s, 0))
                            dump = sb.tile([128, 512], F32, tag="dump", bufs=3)
                            nc.scalar.activation(dump[:isz, :], psu[:isz, :], AF.Exp,
                                                 accum_out=densT[j][:isz, slot:slot + 1])

            # ---------------- quantile stage ----------------
            def quantiles():
                for half, npart in ((0, 128), (1, 64)):
                    s0c = 128 * half
                    cdf = ps.tile([npart, GRID], F32, tag="psu", bufs=3)
                    for cg in range(npart // 32):
                        for j, (i0, isz) in enumerate(ITILES):
                            nc.tensor.matmul(cdf[32 * cg:32 * cg + 32, :],
                                             densT[j][:isz, s0c + 32 * cg:s0c + 32 * cg + 32],
                                             tri[:isz, j, :],
                                             start=(j == 0), stop=(j == 3),
                                             tile_position=(0, 32 * cg))
                    rec = sb.tile([npart, 1], F32, tag="rec")
                    nc.vector.reciprocal(out=rec[:], in_=cdf[:, GRID - 1:GRID])
                    cdfn = sb.tile([npart, GRID], F32, tag="cdfn")
                    nc.vector.tensor_tensor(out=cdfn[:], in0=cdf[:],
                                            in1=rec[:].to_broadcast([npart, GRID]), op=ALU.mult)
                    gp = gridpk[half][0:npart, :]
                    qs = np.linspace(0.0, 1.0, Q)
                    for qi in range(Q):
                        d1 = sb.tile([npart, GRID], F32, tag="d1")
                        nc.vector.tensor_scalar(out=d1[:], in0=cdfn[:],
                                                scalar1=float(-qs[qi]), scalar2=None, op0=ALU.add)
                        nc.vector.tensor_scalar(out=d1[:].bitcast(I32), in0=d1[:].bitcast(I32),
                                                scalar1=0x7FFFFFFF, scalar2=None, op0=ALU.bitwise_and)
                        w = sb.tile([npart, GRID], F32, tag="w", bufs=3)
                        nc.scalar.activation(w[:], d1[:], AF.Sigmoid, scale=-100.0,
                                             accum_out=S0[half][0:npart, qi:qi + 1])
                        gw_t = sb.tile([npart, GRID], F32, tag="gw", bufs=3)
                        nc.vector.tensor_tensor(out=gw_t[:], in0=w[:], in1=gp, op=ALU.mult)
                        nc.scalar.activation(gw_t[:], gw_t[:], AF.Copy,
                                             accum_out=S1[half][0:npart, qi:qi + 1])
                    r0 = sb.tile([npart, Q], F32, tag="r0")
                    nc.vector.tensor_scalar(out=r0[:], in0=S0[half][0:npart, :],
                                            scalar1=1e-8, scalar2=None, op0=ALU.add)
                    nc.vector.reciprocal(out=r0[:], in_=r0[:])
                    qv = sb.tile([npart, Q], F32, tag="qv")
                    nc.vector.tensor_tensor(out=qv[:], in0=S1[half][0:npart, :],
                                            in1=r0[:], op=ALU.mult)
                    nc.sync.dma_start(out=(qv1_out if half == 0 else qv2_out)[:], in_=qv[:])

            # ---------------- GAT layer ----------------
            def finish_tile(l, t, raw, denps):
                rd = sb.tile([128, 4], F32, tag="rd")
                nc.vector.tensor_scalar(out=rd[:], in0=denps[:, 4 * t:4 * t + 4],
                                        scalar1=1e-16, scalar2=None, op0=ALU.add)
                nc.vector.reciprocal(out=rd[:], in_=rd[:])
                o = sb.tile([128, HC], F32, tag="otile")
                nc.vector.tensor_tensor(
                    out=o[:].rearrange("p (h c2) -> p h c2", h=4),
                    in0=raw[:].rearrange("p (h c2) -> p h c2", h=4),
                    in1=rd[:].rearrange("p h -> p h ()").to_broadcast([128, 4, 32]),
                    op=ALU.mult)
                nc.vector.tensor_tensor(out=o[:], in0=o[:], in1=br[l][:], op=ALU.add)
                if l == 0:
                    nc.vector.tensor_scalar(out=o[:], in0=o[:], scalar1=0.0,
                                            scalar2=None, op0=ALU.max)
                pt = ps.tile([128, 128], F32, tag="ps_m", bufs=1)
                nc.tensor.transpose(pt[:], o[:], id128[:])
                nc.vector.tensor_copy(curT[l + 1][:, 128 * t:128 * (t + 1)], pt[:])

            def gat_layer(l):
                cur = curT[l]
                for b in range(4):
                    pxh = ps.tile([128, 512], F32, tag="ps_m", bufs=1)
                    nc.tensor.matmul(pxh[:], W[l][:], cur[:, 512 * b:512 * (b + 1)],
                                     start=True, stop=True)
                    nc.vector.tensor_copy(xhT[:, 512 * b:512 * (b + 1)], pxh[:])
                for t in range(16):
                    pal = ps.tile([128, 8], F32, tag="ps_m", bufs=1)
                    nc.tensor.matmul(pal[:], xhT[:, 128 * t:128 * (t + 1)], Av[l][:],
                                     start=True, stop=True)
                    nc.vector.tensor_copy(al[:, t, :], pal[:])
                    pxr = ps.tile([128, 128], F32, tag="ps_m", bufs=1)
                    nc.tensor.transpose(pxr[:], xhT[:, 128 * t:128 * (t + 1)], id128[:])
                    nc.vector.tensor_copy(stage[:, t, 0:128], pxr[:])
                nc.vector.tensor_copy(alr[:], al[:])
                nc.vector.tensor_copy(stage[:, :, 128:132], al[:, :, 0:4])
                nc.sync.dma_start(out=xh_al_hbm[:].rearrange("(t p) d -> p t d", p=128),
                                  in_=stage[:])
                denps = ps.tile([128, 64], F32, tag="denps")
                raw = None
                cur_t = -1
                for g0 in range(0, C, EGRP):
                    cn = min(EGRP, C - g0)
                    evps = ps.tile([128, 4 * EGRP], F32, tag="evps", bufs=1)
                    for ci in range(cn):
                        c = g0 + ci
                        ohg = sb.tile([128, 128], F32R, tag="ohg", bufs=6)
                        nc.sync.dma_start(out=ohg[:], in_=ohdg_in[c, :, :])
                        nc.tensor.matmul(evps[:, 4 * ci:4 * ci + 4], ohg[:],
                                         alr[:, int(chunk_tile[c]), 4:8],
                                         start=True, stop=True)
                        nc.gpsimd.indirect_dma_start(
                            out=gwin[:, c % GW, :], out_offset=None, in_=xh_al_hbm[:],
                            in_offset=bass.IndirectOffsetOnAxis(ap=sidx[:, c:c + 1], axis=0))
                    r0w = g0 % GW
                    z = sb.tile([128, EGRP, 4], F32, tag="z", bufs=2)
                    nc.vector.tensor_tensor(
                        out=z[:, 0:cn, :],
                        in0=evps[:, 0:4 * cn].rearrange("p (c f) -> p c f", f=4),
                        in1=gwin[:, r0w:r0w + cn, 128:132], op=ALU.add)
                    zs = sb.tile([128, EGRP, 4], F32, tag="zs", bufs=2)
                    nc.vector.tensor_scalar(out=zs[:, 0:cn, :], in0=z[:, 0:cn, :],
                                            scalar1=NEG_SLOPE, scalar2=None, op0=ALU.mult)
                    nc.vector.tensor_tensor(out=z[:, 0:cn, :], in0=z[:, 0:cn, :],
                                            in1=zs[:, 0:cn, :], op=ALU.max)
                    nc.scalar.activation(
                        p_r[:, 4 * g0:4 * (g0 + cn)].rearrange("p (c f) -> p c f", f=4),
                        z[:, 0:cn, :], AF.Exp)
                    for ci in range(cn):
                        c = g0 + ci
                        t = int(chunk_tile[c])
                        first = (c == 0) or (int(chunk_tile[c - 1]) != t)
                        last = (c == C - 1) or (int(chunk_tile[c + 1]) != t)
                        if first:
                            if raw is not None:
                                finish_tile(l, cur_t, raw, denps)
                            raw = ps.tile([128, HC], F32, tag="raw",
                                          padded_shape=[128, 512], bufs=2)
                            cur_t = t
                        ohv = sb.tile([128, 128], F32R, tag="ohv", bufs=6)
                        nc.sync.dma_start(out=ohv[:], in_=ohrev_in[c, :, :])
                        nc.tensor.matmul(denps[:, 4 * t:4 * t + 4], ohv[:],
                                         p_r[:, 4 * c:4 * c + 4],
                                         start=first, stop=last, skip_group_check=True)
                        sxh = sb.tile([128, HC], F32R, tag="sxh", bufs=3)
                        nc.vector.tensor_tensor(
                            out=sxh[:].rearrange("p (h c2) -> p h c2", h=4),
                            in0=gwin[:, c % GW, 0:128].rearrange("p (h c2) -> p h c2", h=4),
                            in1=p_r[:, 4 * c:4 * c + 4].rearrange("p h -> p h ()").to_broadcast([128, 4, 32]),
                            op=ALU.mult)
                        nc.tensor.matmul(raw[:], ohv[:], sxh[:],
                                         start=first, stop=last, skip_group_check=True)
                finish_tile(l, cur_t, raw, denps)
                if KDBG and l == 0:
                    nc.sync.dma_start(out=dbg_p[:], in_=p_r[:].bitcast(F32))
                    dcp = sb.tile([128, 64], F32, tag="dcp")
                    nc.vector.tensor_copy(dcp[:], denps[:])
                    nc.sync.dma_start(out=dbg_den[:], in_=dcp[:])

            # ---------------- main ----------------
            import os
            phases = os.environ.get("KPHASES", "all")
            if phases != "all":
                nc.gpsimd.memset(gridpk[0][:], 0.0)
                nc.gpsimd.memset(gridpk[1][:], 0.0)
                for j in range(4):
                    nc.gpsimd.memset(densT[j][:], 0.0)
                nc.gpsimd.memset(curT[1][:], 0.0)
                nc.gpsimd.memset(curT[2][:], 0.0)
            for _ in range(reps):
                if phases == "r0":
                    readout(0); quantiles()
                elif phases == "r0g0":
                    readout(0); gat_layer(0); quantiles()
                else:
                    readout(0)
                    gat_layer(0)
                    readout(1)
                    gat_layer(1)
                    readout(2)
                    quantiles()
            nc.sync.dma_start(out=pmean_out[:], in_=pmean[:])
            nc.sync.dma_start(out=pmax_out[:], in_=pmax[:])
            if KDBG:
                for j in range(4):
                    nc.sync.dma_start(out=dbg_dens[j][:], in_=densT[j][:])
                nc.sync.dma_start(out=dbg_cur1[:], in_=curT[1][:])
                nc.sync.dma_start(out=dbg_cur2[:], in_=curT[2][:])
    nc.compile()
    return nc


_CACHE = {}


def _get_program(C, chunk_tile, reps=1):
    import os; key = (C, tuple(chunk_tile.tolist()), reps, os.environ.get("KPHASES", "all"), os.environ.get("KDEBUG"))
    if key not in _CACHE:
        _CACHE[key] = build_program(C, chunk_tile, reps)
    return _CACHE[key]


def _host_inputs(inputs, s_idx, C, ohd_g, ohrev):
    x = np.asarray(inputs["x"], np.float32)
    repq = np.zeros((4, 4, DPC, 128), np.float32)
    repx = np.zeros((4, 3, DPC, 128), np.float32)
    for m in range(4):
        for s in range(4):
            k = 4 * m + s
            for r in range(4):
                repq[m, r, k, 32 * s + r] = 1.0
            repx[m, 0, k, 32 * s + 0] = 1.0
            repx[m, 0, k, 32 * s + 3] = 1.0
            repx[m, 1, k, 32 * s + 1] = 1.0
            repx[m, 2, k, 32 * s + 2] = 1.0
    im_base = dict(
        repq=repq, repx=repx,
        xT=np.ascontiguousarray(x.T),
        sidx=s_idx,
        ohdg=ohd_g, ohrev=ohrev,
        iota3=np.stack([np.ones(GRID), np.arange(GRID), np.arange(GRID) ** 2]).astype(np.float32),
        tri=np.triu(np.ones((GRID, GRID), np.float32)),  # U[j,i]=1 if j<=i
    )
    for l in range(2):
        A = np.zeros((128, 8), np.float32)
        as_l = np.asarray(inputs[f"as{l}"], np.float32)
        ad_l = np.asarray(inputs[f"ad{l}"], np.float32)
        for h in range(HEADS):
            A[h * HID:(h + 1) * HID, h] = as_l[h]
            A[h * HID:(h + 1) * HID, 4 + h] = ad_l[h]
        im_base[f"W{l}"] = np.asarray(inputs[f"W{l}"], np.float32)
        im_base[f"Av{l}"] = A
        im_base[f"brep{l}"] = np.tile(np.asarray(inputs[f"b{l}"], np.float32)[None, :], (128, 1))
    in_maps = []
    for c in range(N_CORES):
        selm = np.zeros((128, DPC), np.float32)
        for k in range(DPC):
            selm[DPC * c + k, k] = 1.0
        in_maps.append({**im_base, "sel": selm})
    return in_maps


def kernel(**inputs) -> np.ndarray:
    from concourse.bass_utils import run_bass_kernel_spmd
    s_idx, chunk_tile, ohd_g, ohrev = _edge_prep(np.asarray(inputs["edge_index"]))
    C = s_idx.shape[1]
    nc = _get_program(C, chunk_tile)
    in_maps = _host_inputs(inputs, s_idx, C, ohd_g, ohrev)
    res = run_bass_kernel_spmd(nc, in_maps, list(range(N_CORES))).results
    return _assemble(inputs, res)


def _assemble(inputs, res):
    # kf[l, g, d, q]
    kf = np.zeros((3, G, 128, Q), np.float64)
    for c in range(N_CORES):
        qv = np.concatenate([res[c]["qv1"], res[c]["qv2"]], 0)  # [192, Q]
        for u in range(UNITS):
            l, g = divmod(u, G)
            kf[l, g, DPC * c:DPC * (c + 1), :] = qv[u * DPC:(u + 1) * DPC, :]
    pmean = res[0]["pmean"] / 512.0          # [128, 12]
    pmax = res[0]["pmax"]
    pool_w = np.asarray(inputs["pool_w"], np.float64)
    beta = np.asarray(inputs["beta"], np.float64)
    h0 = float(np.asarray(inputs["h0"]).reshape(-1)[0])
    h_list, k_list = [], []
    for l in range(3):
        wp = (pool_w[0] * pmean[:, l * G:(l + 1) * G] + pool_w[1] * pmax[:, l * G:(l + 1) * G]).T  # [G, 128]
        lpW = np.asarray(inputs[f"lpW{l}"], np.float64)
        lpb = np.asarray(inputs[f"lpb{l}"], np.float64)
        h_list.append(wp @ lpW + lpb)
        kW = np.asarray(inputs[f"kW{l}"], np.float64)
        kb = np.asarray(inputs[f"kb{l}"], np.float64)
        k_list.append(kf[l].reshape(G, 128 * Q) @ kW + kb)
    main_out = np.mean(h_list, axis=0)
    kde_out = np.mean(k_list, axis=0)
    risk = (main_out + kde_out) @ beta + h0
    return risk.astype(np.float32)
